# revision 50
# baseline (speedup 1.0000x reference)
"""DeformableAttention1D on 8 TRN2 NeuronCores via Bass/Tile.

Sharding: core c handles offset-group g=c//2 (64 of 256 channels, 2 of 8 heads)
and query-half qh=c%2 (512 of 1024 positions). Each core computes its group's
offsets/gather/CPB/attention independently; the final output projection is
computed as a partial (wo sliced by group) and summed on the host (the
"all-reduce" of the output projection).

CPB bias via a 1-D table: bias[h,i,j] = F_h(grid_q[i] - vgrid[j]) where F_h is
the scalar CPB MLP. grid_q is uniform with step d=2/1023, so with a table
T_h[m] = F_h((m-1088)*d) the bias row for query block i is the contiguous
slice T_h[k_j + i] lerped with a per-j fraction w_j. The slice gather uses
dma_gather over a 64-phase replicated copy of T in DRAM (row stride must be
a multiple of 256B); the lerp is two diagonal-matrix PE matmuls accumulating
straight into the attention-logit PSUM together with q@k.

The kv deformable gather also uses dma_gather: the host passes x transposed
(xgT [1025, 64], row 1024 zeros); rows i0/i1 are gathered per j (indices
clamped via an unsigned-min to the zero row, which reproduces the reference's
zero padding), then lerped with per-partition weights and PE-transposed back.

The ACT engine is restricted to ONE table set (natural_log_exp_and_others:
Exp/Ln/Relu/Copy/Identity/Square) because runtime table swaps are broken in
this environment; tanh and erf(gelu) are composed from Exp + DVE ops.
"""
import os
import sys

sys.path.insert(0, "/opt/trn_rl_repo")

DEBUG = bool(os.environ.get("DEFORM_DEBUG"))

import numpy as np

import concourse.bacc as bacc
import concourse.bass as bass
import concourse.mybir as mybir
import concourse.tile as tile
import concourse.bass_utils as bass_utils
from concourse.ap import AP

F32 = mybir.dt.float32
F32R = mybir.dt.float32r
I32 = mybir.dt.int32
I16 = mybir.dt.int16
U32 = mybir.dt.uint32
BF16 = mybir.dt.bfloat16
AF = mybir.ActivationFunctionType
ALU = mybir.AluOpType

# model dims (hardcoded per problem spec)
DIM = 256
N = 1024
G = 4
HEADS = 8
DH = 32
NDS = 256          # downsampled kv positions
QS = 512           # queries per core
DPG = 64           # channels per group
OFF_K = 6
DS = 4             # downsample stride
OFF_SCALE = 4.0
NCORES = 8

# CPB bias table (bf16: 128-elem row stride = 256B)
OFFT = 1088        # table center: u = (tm - OFFT) * DELTA
MTAB = 2176        # table points
TPAD = 2304        # padded table length per head in DRAM
RSPAN = 2176       # elements per phase copy; rows per head = 17*128 = 2176
GROW = 640         # gathered row length (513 needed, 640 bf16 = 1280B)
NRROWS = 4348      # rows addressable in R (128-elem stride)
DELTA = 2.0 / 1023.0

# A&S 7.1.25 erf coefficients (3-term, |err| <= 2.5e-5)
ERF_P3 = 0.47047
ERF_A3 = [0.3480242, -0.0958798, 0.7478556]

_CACHED = {}


def _patch_act_tables():
    """Restrict activation-table selection to the single set that covers all
    ACT functions used by this kernel, so exactly one table load is emitted
    (runtime table swaps do not work in this environment)."""
    import concourse.hw_specs as hw_specs

    if getattr(bacc, "_deform_act_patch", False):
        return
    orig = hw_specs.get_activation_tables

    keep = "natural_log_exp_and_others"

    def patched(module_arch):
        tabs = orig(module_arch)
        keep_funcs = tabs[keep]
        out = {}
        for name, funcs in tabs.items():
            if name == keep:
                out[name] = funcs
            else:
                out[name] = funcs - keep_funcs
        return out

    bacc.get_activation_tables = patched
    bacc._deform_act_patch = True


def _erf_gelu(nc, sb, out_ap, x_ap, shape):
    """out = x * (1 + erf(x/sqrt(2))) via A&S 7.1.25 (0.5 folded into wproj)."""
    P, Nf = shape
    sq = sb.tile([P, Nf], F32, name="gelu_sq", tag="gelu_sq")
    nc.scalar.activation(sq[:], x_ap, AF.Square)
    e = sb.tile([P, Nf], F32, name="gelu_e", tag="gelu_e")
    # e = exp(-x^2/2)
    nc.scalar.activation(e[:], sq[:], AF.Exp, scale=-0.5)
    ax = sb.tile([P, Nf], F32, name="gelu_ax", tag="gelu_ax")
    # |x|/sqrt(2) = max(x, -x) * (1/sqrt2) folded into t below
    nc.vector.scalar_tensor_tensor(ax[:], x_ap, -1.0, x_ap, ALU.mult, ALU.max)
    t = sb.tile([P, Nf], F32, name="gelu_t", tag="gelu_t")
    nc.vector.tensor_scalar(t[:], ax[:], float(ERF_P3 / np.sqrt(2.0)), 1.0, ALU.mult, ALU.add)
    nc.vector.reciprocal(t[:], t[:])
    poly = sb.tile([P, Nf], F32, name="gelu_poly", tag="gelu_poly")
    # P(t) = ((a3 t + a2) t + a1) t
    nc.vector.tensor_scalar(poly[:], t[:], ERF_A3[2], ERF_A3[1], ALU.mult, ALU.add)
    nc.vector.tensor_tensor(poly[:], poly[:], t[:], ALU.mult)
    nc.vector.scalar_tensor_tensor(poly[:], poly[:], ERF_A3[0], t[:], ALU.add, ALU.mult)
    # erfa = 1 - poly*e   (= erf(|x|/sqrt2))
    erfa = sb.tile([P, Nf], F32, name="gelu_erfa", tag="gelu_erfa")
    nc.vector.tensor_tensor(erfa[:], poly[:], e[:], ALU.mult)
    nc.vector.tensor_scalar(erfa[:], erfa[:], -1.0, 1.0, ALU.mult, ALU.add)
    # copysign: erf(x) = sign(x)*erfa
    sgn = sb.tile([P, Nf], U32, name="gelu_sgn", tag="gelu_sgn")
    nc.vector.tensor_scalar(sgn[:], x_ap.bitcast(U32), 0x80000000, None, ALU.bitwise_and)
    erfs = sb.tile([P, Nf], F32, name="gelu_erfs", tag="gelu_erfs")
    nc.vector.tensor_tensor(erfs[:].bitcast(U32), erfa[:].bitcast(U32), sgn[:], ALU.bitwise_or)
    # out = (1 + erf) * x
    nc.vector.tensor_scalar(erfs[:], erfs[:], 1.0, None, ALU.add)
    nc.vector.tensor_tensor(out_ap, erfs[:], x_ap, ALU.mult)


def _tanh_rows(nc, sb, out_ap, x_ap, shape):
    """out = tanh(x) = sign(x) * (1 - 2/(exp(2*min(|x|,30))+1)) on small tiles."""
    P, Nf = shape
    ax = sb.tile([P, Nf], F32, name="th_ax", tag="th_ax")
    nc.vector.scalar_tensor_tensor(ax[:], x_ap, -1.0, x_ap, ALU.mult, ALU.max)
    nc.vector.tensor_scalar(ax[:], ax[:], 30.0, None, ALU.min)
    e = sb.tile([P, Nf], F32, name="th_e", tag="th_e")
    nc.scalar.activation(e[:], ax[:], AF.Exp, scale=2.0)
    nc.vector.tensor_scalar(e[:], e[:], 1.0, None, ALU.add)
    r = sb.tile([P, Nf], F32, name="th_r", tag="th_r")
    nc.vector.reciprocal(r[:], e[:])
    nc.vector.tensor_scalar(r[:], r[:], -2.0, 1.0, ALU.mult, ALU.add)
    sgn = sb.tile([P, Nf], U32, name="th_sgn", tag="th_sgn")
    nc.vector.tensor_scalar(sgn[:], x_ap.bitcast(U32), 0x80000000, None, ALU.bitwise_and)
    nc.vector.tensor_tensor(out_ap.bitcast(U32), r[:].bitcast(U32), sgn[:], ALU.bitwise_or)


def build_nc():
    _patch_act_tables()
    nc = bacc.Bacc("TRN2", target_bir_lowering=False, debug=False, num_devices=NCORES)

    din = {}

    def dt_in(name, shape):
        din[name] = nc.dram_tensor(name, shape, F32, kind="ExternalInput")
        return din[name]

    dt_in("xg", [DPG, N])
    dt_in("xq", [DPG, QS])
    dt_in("xgT", [N + 1, DPG])     # x group transposed; row N (=1024) is zeros
    dt_in("packed", [128, 790])
    y_out = nc.dram_tensor("y", [DIM, QS], F32, kind="ExternalOutput")
    dbg = {}
    if DEBUG:
        for nm, shp in [("dbg_q", [DPG, N]), ("dbg_ridx", [1, NDS]),
                        ("dbg_kv", [DPG, NDS]),
                        ("dbg_k", [DPG, NDS]), ("dbg_v", [DPG, NDS]),
                        ("dbg_T", [4, 1088]), ("dbg_S0", [128, 4 * GROW]),
                        ("dbg_avn", [DPG, QS])]:
            dbg[nm] = nc.dram_tensor(nm, shp, F32, kind="ExternalOutput")

    with tile.TileContext(nc) as tc:
        with (
            tc.tile_pool(name="const", bufs=1) as cst,
            tc.tile_pool(name="work", bufs=2) as wk,
            tc.tile_pool(name="rows", bufs=1) as rw,
            tc.tile_pool(name="persist", bufs=1) as pe_pool,
            tc.tile_pool(name="dramt", bufs=1, space="DRAM") as drp,
        ):
            # ---- input DMAs: packed first (gates the table build) ----
            packed = cst.tile([128, 790], F32, name="packed", tag="packed")
            nc.sync.dma_start(packed[:], din["packed"].ap())
            xg = cst.tile([DPG, N], F32, name="xg", tag="xg")
            nc.sync.dma_start(xg[:], din["xg"].ap())
            xq = cst.tile([DPG, QS], F32, name="xq", tag="xq")
            nc.sync.dma_start(xq[:], din["xq"].ap())
            w2bd = packed[:, 0:128]
            eyet = packed[:, 128:256]
            wqT = packed[0:DPG, 256:320]
            wqTs = packed[0:DPG, 320:384]
            wkT = packed[0:DPG, 384:448]
            wvT = packed[0:DPG, 448:512]
            woT = packed[0:DPG, 512:768]
            wdw = packed[0:DPG, 768:774]
            bodw = packed[0:DPG, 774:775]
            wproj_half = packed[0:DPG, 775:776]
            b1col = packed[:, 776:777]
            b2col = packed[:, 777:778]
            qbofft = packed[:, 778:779]     # OFFT + 512*qh
            w3bd = packed[:, 781:785]
            w1dup = packed[:, 785:786]      # [w1; w1] column
            pofs = packed[:, 786:787]       # -1088*(p<64), 0*(p>=64) column

            # ---- small constants / engine warmup ----
            ones_col = cst.tile([128, 1], F32, name="ones", tag="ones")
            nc.gpsimd.memset(ones_col[:], 1.0)
            warm = cst.tile([128, 1], F32, name="warm", tag="warm")
            nc.scalar.activation(warm[:], ones_col[:], AF.Relu)
            ones_colr = cst.tile([128, 1], F32R, name="onesr", tag="onesr")
            nc.vector.tensor_copy(ones_colr[:], ones_col[:])
            ones_rf = cst.tile([1, 32], F32, name="onesrf", tag="onesrf")
            nc.gpsimd.memset(ones_rf[:], 1.0)
            ones_rowr = cst.tile([1, 32], F32R, name="onesrw", tag="onesrw")
            nc.vector.tensor_copy(ones_rowr[:], ones_rf[:])

            # persistent tiles
            k_sb = pe_pool.tile([DPG, NDS], F32R, name="k_sb", tag="k_sb")
            qs_sb = pe_pool.tile([DPG, QS], F32R, name="qs_sb", tag="qs_sb")
            vT = [pe_pool.tile([128, DPG], F32R, name=f"vT{H}", tag=f"vT{H}") for H in range(2)]
            avn = pe_pool.tile([DPG, QS], F32R, name="avn", tag="avn")
            gath = pe_pool.tile([128, 4 * GROW], BF16, name="gath", tag="gath")
            gkv = pe_pool.tile([128, 4 * DPG], F32, name="gkv", tag="gkv")
            diag_w = [pe_pool.tile([128, 128], BF16, name=f"diag_w{H}", tag=f"diag_w{H}")
                      for H in range(2)]
            diag_1w = [pe_pool.tile([128, 128], BF16, name=f"diag_1w{H}", tag=f"diag_1w{H}")
                       for H in range(2)]

            t_dram = drp.tile([2, TPAD], BF16, name="t_dram", tag="t_dram")
            r_dram = drp.tile([2, 128 * RSPAN], BF16, name="r_dram", tag="r_dram")

            # ================= CPB table build (offset-independent) =========
            with (
                tc.tile_pool(name="tblw", bufs=1) as tbw,
                tc.tile_pool(name="psT", bufs=1, space="PSUM") as psT,
                tc.tile_pool(name="psT2", bufs=1, space="PSUM") as psT2,
                tc.tile_pool(name="psA", bufs=2, space="PSUM") as psA,
                tc.tile_pool(name="psA1", bufs=1, space="PSUM") as psA1,
            ):
                # --- phi(u) in [128 = 2x64 hidden, 1088] broadcast layout ---
                io_t = tbw.tile([128, 1088], F32, name="io_t", tag="io_t")
                nc.gpsimd.iota(io_t[:], pattern=[[1, 1088]], base=0, channel_multiplier=0,
                               allow_small_or_imprecise_dtypes=True)
                xgr = cst.tile([DPG, N], F32R, name="xgr", tag="xgr")
                nc.vector.tensor_copy(xgr[:], xg[:])
                xqr = cst.tile([DPG, QS], F32R, name="xqr", tag="xqr")
                nc.vector.tensor_copy(xqr[:], xq[:])
                wqTr = cst.tile([DPG, DPG], F32R, name="wqTr", tag="wqTr")
                nc.vector.tensor_copy(wqTr[:], wqT)
                wqTsr = cst.tile([DPG, DPG], F32R, name="wqTsr", tag="wqTsr")
                nc.vector.tensor_copy(wqTsr[:], wqTs)
                u_t = tbw.tile([128, 1088], F32, name="u_t", tag="u_t")
                nc.vector.tensor_scalar(u_t[:], io_t[:], pofs, DELTA, ALU.add, ALU.mult)
                au_t = tbw.tile([128, 1088], F32, name="au_t", tag="au_t")
                nc.vector.scalar_tensor_tensor(au_t[:], u_t[:], -1.0, u_t[:], ALU.mult, ALU.max)
                sg_t = tbw.tile([128, 1088], U32, name="sg_t", tag="sg_t")
                nc.vector.tensor_scalar(sg_t[:], u_t[:].bitcast(U32), 0x80000000, None, ALU.bitwise_and)
                # f32r weight copies (DVE, cheap, off the phi critical path)
                w2bdr = cst.tile([128, 128], F32R, name="w2bdr", tag="w2bdr")
                nc.vector.tensor_copy(w2bdr[:], w2bd)
                w3bdr = cst.tile([128, 4], F32R, name="w3bdr", tag="w3bdr")
                nc.vector.tensor_copy(w3bdr[:], w3bd)
                woTr = cst.tile([DPG, DIM], F32R, name="woTr", tag="woTr")
                nc.vector.tensor_copy(woTr[:], woT)
                wkTr = cst.tile([DPG, DPG], F32R, name="wkTr", tag="wkTr")
                nc.vector.tensor_copy(wkTr[:], wkT)
                wvTr = cst.tile([DPG, DPG], F32R, name="wvTr", tag="wvTr")
                nc.vector.tensor_copy(wvTr[:], wvT)
                ln_t = tbw.tile([128, 1088], F32, name="ln_t", tag="ln_t")
                nc.scalar.activation(ln_t[:], au_t[:], AF.Ln, bias=1.0)
                phi_t = tbw.tile([128, 1088], F32, name="phi_t", tag="phi_t")
                nc.vector.tensor_tensor(phi_t[:].bitcast(U32), ln_t[:].bitcast(U32), sg_t[:], ALU.bitwise_or)
                h1_t = tbw.tile([128, 1088], F32R, name="h1_t", tag="h1_t")
                nc.scalar.activation(h1_t[:], phi_t[:], AF.Relu, bias=b1col, scale=w1dup)

                # --- q matmuls early on PE (before the table MLP matmuls) ---
                # q packed: q_pad[c + 64h, t] = q[c, 512h + t - 1]
                q_pad = pe_pool.tile([128, 516], F32, name="q_pad", tag="q_pad")
                nc.gpsimd.memset(q_pad[:], 0.0)
                pqh = []
                for h in range(2):
                    pq = psA.tile([DPG, QS], F32, name="pA512", tag="pA512")
                    nc.tensor.matmul(pq[:], wqTr[:], xgr[:, h * QS:(h + 1) * QS])
                    nc.scalar.copy(q_pad[64 * h:64 * (h + 1), 1:513], pq[:])
                    pqh.append(pq)
                nc.scalar.copy(q_pad[0:64, 513:514], pqh[1][:, 0:1])
                nc.scalar.copy(q_pad[64:128, 0:1], pqh[0][:, 511:512])
                pqs = psA.tile([DPG, QS], F32, name="pA512", tag="pA512")
                nc.tensor.matmul(pqs[:], wqTsr[:], xqr[:])
                nc.scalar.copy(qs_sb[:], pqs[:])

                # --- table MLP layers 2+3 in 512-col blocks ---
                t_sb = tbw.tile([4, 1088], BF16, name="t_sb", tag="t_sb")
                for blo, bhi in ((0, 512), (512, 1024), (1024, 1088)):
                    bw = bhi - blo
                    pre2 = psT.tile([128, bw], F32, name="pre2", tag="pre2")
                    nc.tensor.matmul(pre2[:], w2bdr[:], h1_t[:, blo:bhi])
                    h2_t = wk.tile([128, bw], F32R, name="h2_t", tag="h2_t")
                    nc.scalar.activation(h2_t[:], pre2[:], AF.Relu, bias=b2col)
                    pt3 = psT2.tile([4, bw], F32, name="pt3", tag="pt3")
                    nc.tensor.matmul(pt3[:], w3bdr[:], h2_t[:])
                    nc.scalar.copy(t_sb[:, blo:bhi], pt3[:])
                # T rows (2*half + o) -> t_dram[o, 1088*half + q]; zero pad tail
                zrow = tbw.tile([1, 128], F32, name="zrow", tag="zrow")
                nc.gpsimd.memset(zrow[:], 0.0)
                for o in range(2):
                    # w3bd cols reordered so t_sb row 2o+half = (o, half)
                    nc.sync.dma_start(t_dram[:][o, 0:MTAB].rearrange("(h q) -> h q", h=2),
                                      t_sb[2 * o:2 * o + 2, :])
                    nc.sync.dma_start(t_dram[:][o, MTAB:TPAD],
                                      zrow[:, 0:(TPAD - MTAB) // 2].bitcast(BF16))
                # R: 64 overlapping phase copies per head, one 3-dim DMA
                tv = t_dram[:]
                rsrc = AP(tv.tensor, tv.offset, [[TPAD, 2], [1, 128], [1, RSPAN]])
                nc.sync.dma_start(r_dram[:].rearrange("o (c e) -> o c e", c=128), rsrc)
                if DEBUG:
                    nc.sync.dma_start(dbg["dbg_T"].ap(), t_sb[:].bitcast(F32))

                # ===== offsets chain (packed: [128, 128], rows [2, 128]) =====
                # depthwise strided conv; partition block h covers j in
                # [128h, 128h+128); tap kk reads q_pad[:, kk + 4*jj]
                wdw_d = packed[:, 768:774]
                bodw_d = packed[:, 774:775]
                wproj2 = packed[:, 787:789]
                acc = wk.tile([128, 128], F32, name="conv_acc", tag="conv_acc")
                acc2 = wk.tile([128, 128], F32, name="conv_acc2", tag="conv_acc2")
                nc.vector.tensor_scalar(
                    acc[:], q_pad[:, 0:509:DS], wdw_d[:, 0:1], bodw_d, ALU.mult, ALU.add)
                nc.vector.tensor_scalar(
                    acc2[:], q_pad[:, 1:510:DS], wdw_d[:, 1:2], None, ALU.mult)
                for kk in range(2, OFF_K):
                    dst = acc if kk % 2 == 0 else acc2
                    nc.vector.scalar_tensor_tensor(
                        dst[:], q_pad[:, kk:kk + 509:DS], wdw_d[:, kk:kk + 1], dst[:],
                        ALU.mult, ALU.add)
                nc.vector.tensor_tensor(acc[:], acc[:], acc2[:], ALU.add)

                gl = wk.tile([128, 128], F32, name="gelu_out", tag="gelu_out")
                _erf_gelu(nc, wk, gl[:], acc[:], [128, 128])

                # proj rows: pproj[h, jj] = sum_c 0.5*wproj[c]*gl[c+64h, jj]
                pproj = psA1.tile([2, 128], F32, name="pproj", tag="pproj")
                nc.tensor.matmul(pproj[:], wproj2, gl[:])
                proj_sb = rw.tile([2, 128], F32, name="proj_sb", tag="proj_sb")
                nc.scalar.copy(proj_sb[:], pproj[:])
                th = rw.tile([2, 128], F32, name="th", tag="th")
                _tanh_rows(nc, rw, th[:], proj_sb[:], [2, 128])

                iotaj = rw.tile([2, 128], F32, name="iotaj", tag="iotaj")
                nc.gpsimd.iota(iotaj[:], pattern=[[1, 128]], base=0, channel_multiplier=128,
                               allow_small_or_imprecise_dtypes=True)
                vgrid = rw.tile([2, 128], F32, name="vgrid", tag="vgrid")
                nc.vector.scalar_tensor_tensor(vgrid[:], th[:], OFF_SCALE, iotaj[:], ALU.mult, ALU.add)

                # ---- kv gather index chain (Pool helps, parallel with CPB) ----
                ppix = rw.tile([2, 128], F32, name="ppix", tag="ppix")
                nc.vector.tensor_scalar(ppix[:], vgrid[:], float(N / (NDS - 1)), -0.5, ALU.mult, ALU.add)
                pi = rw.tile([2, 128], I32, name="pi", tag="pi")
                nc.vector.tensor_copy(pi[:], ppix[:])
                pc = rw.tile([2, 128], F32, name="pc", tag="pc")
                nc.vector.tensor_copy(pc[:], pi[:])
                i01 = rw.tile([2, 256], F32, name="i01", tag="i01")
                w0r = rw.tile([2, 128], F32, name="w0r", tag="w0r")
                w1r = rw.tile([2, 128], F32, name="w1r", tag="w1r")
                gtp = rw.tile([2, 128], F32, name="gtp", tag="gtp")
                i0f = rw.tile([2, 128], F32, name="i0f", tag="i0f")
                nc.vector.tensor_tensor(gtp[:], pc[:], ppix[:], ALU.is_gt)
                nc.gpsimd.tensor_sub(i0f[:], pc[:], gtp[:])
                nc.gpsimd.tensor_sub(w1r[:], ppix[:], i0f[:])
                nc.vector.tensor_scalar(w0r[:], w1r[:], -1.0, 1.0, ALU.mult, ALU.add)
                # clamp OOB to the zero row (1024): unsigned-min on f32 bits
                nc.gpsimd.tensor_scalar_min(i01[:, 0:128].bitcast(U32), i0f[:].bitcast(U32),
                                            0x44800000)
                i1f = rw.tile([2, 128], F32, name="i1f", tag="i1f")
                nc.gpsimd.tensor_scalar_add(i1f[:], i0f[:], 1.0)
                nc.gpsimd.tensor_scalar_min(i01[:, 128:256].bitcast(U32), i1f[:].bitcast(U32),
                                            0x44800000)

                # ---- CPB table index chain (DVE) ----
                ridx = rw.tile([2, 128], F32, name="ridx", tag="ridx")
                nc.vector.tensor_scalar(ridx[:], vgrid[:], float(-1023.0 / 255.0), qbofft[0:2, 0:1],
                                        ALU.mult, ALU.add)
                ki = rw.tile([2, 128], I32, name="ki", tag="ki")
                nc.vector.tensor_copy(ki[:], ridx[:])
                kc = rw.tile([2, 128], F32, name="kc", tag="kc")
                nc.vector.tensor_copy(kc[:], ki[:])
                gtk = rw.tile([2, 128], F32, name="gtk", tag="gtk")
                nc.vector.tensor_tensor(gtk[:], kc[:], ridx[:], ALU.is_gt)
                kf = rw.tile([2, 128], F32, name="kf", tag="kf")
                nc.vector.tensor_tensor(kf[:], kc[:], gtk[:], ALU.subtract)
                wfr = rw.tile([2, 128], F32, name="wfr", tag="wfr")
                nc.vector.tensor_tensor(wfr[:], ridx[:], kf[:], ALU.subtract)
                kii = rw.tile([2, 128], I32, name="kii", tag="kii")
                nc.vector.tensor_copy(kii[:], kf[:])
                # r = 17*(k & 127) + (k >> 7), +RSPAN rows for head 1
                q64 = rw.tile([2, 128], I32, name="q64", tag="q64")
                nc.vector.tensor_scalar(q64[:], kii[:], 7, None, ALU.arith_shift_right)
                cph = rw.tile([2, 128], I32, name="cph", tag="cph")
                nc.vector.tensor_scalar(cph[:], kii[:], 127, None, ALU.bitwise_and)
                ri = rw.tile([2, 128], I32, name="ri", tag="ri")
                nc.vector.scalar_tensor_tensor(ri[:], cph[:], 17, q64[:], ALU.mult, ALU.add)
                rb0 = rw.tile([2, 128], F32, name="rb0", tag="rb0")
                nc.vector.tensor_copy(rb0[:], ri[:])
                rb1 = rw.tile([2, 128], F32, name="rb1", tag="rb1")
                nc.vector.tensor_scalar(rb1[:], rb0[:], float(RSPAN), None, ALU.add)

                # ---- lerp weights to per-partition columns (one PE transpose each) ----
                def cols2(row2_ap, nm):
                    ptv = psA.tile([128, 128], F32, name=f"ptv{nm}", tag="ptp")
                    nc.tensor.transpose(ptv[:, 0:2], row2_ap, eyet[0:2, 0:2])
                    col = rw.tile([128, 2], F32, name=f"c{nm}", tag=f"c{nm}")
                    nc.vector.tensor_copy(col[:], ptv[:, 0:2])
                    return col

                w0c2 = cols2(w0r[:], "w0")
                w1c2 = cols2(w1r[:], "w1")
                wfc2 = cols2(wfr[:], "wf")
                w0c = [w0c2[:, 0:1], w0c2[:, 1:2]]
                w1c = [w1c2[:, 0:1], w1c2[:, 1:2]]
                for H in range(2):
                    nc.vector.tensor_scalar(diag_w[H][:], eyet, wfc2[:, H:H + 1], None, ALU.mult)
                    w1m = rw.tile([128, 1], F32, name=f"w1m{H}", tag=f"w1m{H}")
                    nc.vector.tensor_scalar(w1m[:], wfc2[:, H:H + 1], -1.0, 1.0, ALU.mult, ALU.add)
                    nc.vector.tensor_scalar(diag_1w[H][:], eyet, w1m[:], None, ALU.mult)

                # ---- wrap index sets [2, 128]x2 -> [32, 16] -> [32, 128]
                # (8x replicated) -> PE transpose -> [128, 32] -> int16; the
                # two DMAs per set ride the ACT hwdge queue
                def wrap_a(srcs, nm):
                    sw16 = rw.tile([32, 16], F32, name=f"sw16{nm}", tag=f"sw16{nm}")
                    for b, s in enumerate(srcs):
                        nc.scalar.dma_start(sw16[16 * b:16 * (b + 1), :],
                                            s.rearrange("p (s e) -> p s e", s=8))
                    sw = rw.tile([32, 128], F32, name=f"sw{nm}", tag=f"sw{nm}")
                    s16 = sw16[:]
                    sbc = AP(s16.tensor, s16.offset, [list(s16.ap[0]), [0, 8], [1, 16]])
                    nc.scalar.dma_start(sw[:].rearrange("p (r e) -> p r e", r=8), sbc)
                    return sw

                sw_kv = wrap_a([i01[:, 0:128], i01[:, 128:256]], "kv")
                sw_cpb = wrap_a([rb0[:], rb1[:]], "cpb")

                def wrap_b(sw, nm):
                    ptw = psA.tile([128, 128], F32, name=f"ptw{nm}", tag="ptp")
                    nc.tensor.transpose(ptw[:, 0:32], sw[:], eyet[0:32, 0:32])
                    rwp = rw.tile([128, 32], F32, name=f"rw{nm}", tag=f"rw{nm}")
                    nc.vector.tensor_copy(rwp[:], ptw[:, 0:32])
                    ix = rw.tile([128, 32], I16, name=f"ix{nm}", tag=f"ix{nm}")
                    nc.vector.tensor_copy(ix[:], rwp[:])
                    return ix

                ix_kv = wrap_b(sw_kv, "kv")
                ix_cpb = wrap_b(sw_cpb, "cpb")

                # ---- the two gathers ----
                xtv = din["xgT"].ap().flatten()
                ksrc = AP(xtv.tensor, xtv.offset, [[64, N + 1], [1, DPG]])
                nc.gpsimd.dma_gather(gkv[:].rearrange("p (b e) -> p b e", b=4), ksrc,
                                     ix_kv[:], 2 * NDS, 2 * NDS, DPG, elem_step=64)
                rv = r_dram[:].flatten()
                gsrc = AP(rv.tensor, rv.offset, [[128, NRROWS], [1, GROW]])
                nc.gpsimd.dma_gather(gath[:].rearrange("p (b e) -> p b e", b=4), gsrc,
                                     ix_cpb[:], 2 * NDS, 2 * NDS, GROW, elem_step=128)

                # ---- kv lerp + transpose back to [c, j] ----
                kvTw = wk.tile([128, 128], F32, name="kvTw", tag="kvTw")
                for H in range(2):
                    tmp = wk.tile([128, DPG], F32, name="kvt_t", tag="kvt_t")
                    nc.vector.tensor_scalar(tmp[:], gkv[:, DPG * H:DPG * (H + 1)],
                                            w0c[H], None, ALU.mult)
                    nc.vector.scalar_tensor_tensor(kvTw[:, DPG * H:DPG * (H + 1)],
                                                   gkv[:, 2 * DPG + DPG * H:2 * DPG + DPG * (H + 1)],
                                                   w1c[H], tmp[:], ALU.mult, ALU.add)
                kv = wk.tile([DPG, NDS], F32R, name="kv", tag="kv")
                for H in range(2):
                    ptk = psA.tile([128, 128], F32, name="ptk", tag="ptp")
                    nc.tensor.transpose(ptk[0:DPG, :], kvTw[:, DPG * H:DPG * (H + 1)],
                                        eyet)
                    nc.vector.tensor_copy(kv[:, 128 * H:128 * (H + 1)], ptk[0:DPG, :])
                if DEBUG:
                    nc.sync.dma_start(dbg["dbg_kv"].ap(), kv[:].bitcast(F32))

                pk = psA1.tile([DPG, NDS], F32, name="pA256", tag="pA256")
                nc.tensor.matmul(pk[:], wkTr[:], kv[:])
                nc.scalar.copy(k_sb[:], pk[:])
                pv = psA1.tile([DPG, NDS], F32, name="pA256", tag="pA256")
                nc.tensor.matmul(pv[:], wvTr[:], kv[:])
                v_sb = wk.tile([DPG, NDS], F32, name="v_sb", tag="v_sb")
                nc.scalar.copy(v_sb[:], pv[:])
                if DEBUG:
                    nc.sync.dma_start(dbg["dbg_k"].ap(), k_sb[:].bitcast(F32))
                    nc.sync.dma_start(dbg["dbg_v"].ap(), v_sb[:])

                for H in range(2):
                    pt = psA.tile([128, 128], F32, name="ptvv", tag="ptp")
                    nc.tensor.transpose(pt[:, 0:DPG], v_sb[:, H * 128:(H + 1) * 128], eyet[0:DPG, 0:DPG])
                    nc.vector.tensor_copy(vT[H][:], pt[:, 0:DPG])

            # ============ attention ============
            with (
                tc.tile_pool(name="psE", bufs=2, space="PSUM") as psE,
                tc.tile_pool(name="psE1", bufs=1, space="PSUM") as psE1,
                tc.tile_pool(name="psE2", bufs=1, space="PSUM") as psE2,
            ):
                for h in range(2):
                    expT = []
                    for H in range(2):
                        psim = psE.tile([128, QS], F32, name="psim", tag="psim")
                        g0 = gath[:, (2 * h + H) * GROW:(2 * h + H) * GROW + 512]
                        g1 = gath[:, (2 * h + H) * GROW + 1:(2 * h + H) * GROW + 513]
                        nc.tensor.matmul(psim[:], diag_1w[H][:], g0, start=True, stop=False)
                        nc.tensor.matmul(psim[:], diag_w[H][:], g1, start=False, stop=False)
                        nc.tensor.matmul(
                            psim[:], k_sb[32 * h:32 * (h + 1), H * 128:(H + 1) * 128],
                            qs_sb[32 * h:32 * (h + 1), :], start=False, stop=True)
                        et = wk.tile([128, QS], F32R, name="expT", tag="expT")
                        nc.scalar.activation(et[:], psim[:], AF.Exp)
                        expT.append(et)

                    psum_s = psE1.tile([1, QS], F32, name="psum_s", tag="psum_s")
                    for H in range(2):
                        nc.tensor.matmul(psum_s[:], ones_colr[:], expT[H][:],
                                         start=(H == 0), stop=(H == 1))
                    rs = rw.tile([1, QS], F32R, name="rs", tag="rs")
                    with nc.allow_low_precision(reason="f32r 1/sum feeds f32r PE broadcast"):
                        nc.vector.reciprocal(rs[:], psum_s[:])
                    # broadcast 1/sum to 32 partitions via PE (K=1 matmul)
                    prsb = psE1.tile([32, QS], F32, name="prsb", tag="prsb")
                    nc.tensor.matmul(prsb[:], ones_rowr[:], rs[:])

                    pav = psE2.tile([32, QS], F32, name="pav", tag="pav")
                    for H in range(2):
                        nc.tensor.matmul(pav[:], vT[H][:, 32 * h:32 * (h + 1)], expT[H][:],
                                         start=(H == 0), stop=(H == 1))
                    # pav -> SBUF early (ACT, off the recip chain); avn then has
                    # only one PSUM operand (prsb)
                    pav_sb = wk.tile([32, QS], F32, name="pav_sb", tag="pav_sb")
                    nc.scalar.copy(pav_sb[:], pav[:])
                    nc.vector.tensor_tensor(avn[32 * h:32 * (h + 1), :], pav_sb[:], prsb[:], ALU.mult)
                if DEBUG:
                    nc.sync.dma_start(dbg["dbg_avn"].ap(), avn[:].bitcast(F32))

                for m in range(2):
                    py = psE.tile([128, QS], F32, name="py", tag="py")
                    nc.tensor.matmul(py[:], woTr[0:32, m * 128:(m + 1) * 128], avn[0:32, :],
                                     start=True, stop=False)
                    nc.tensor.matmul(py[:], woTr[32:64, m * 128:(m + 1) * 128], avn[32:64, :],
                                     start=False, stop=True)
                    y_sb = wk.tile([128, QS], F32, name="y_sb", tag="y_sb")
                    nc.scalar.copy(y_sb[:], py[:])
                    nc.sync.dma_start(y_out.ap()[m * 128:(m + 1) * 128, :], y_sb[:])

    nc.compile()
    return nc


def _shard_inputs(inputs):
    """Build the 8 per-core input maps from the full inputs."""
    x = np.ascontiguousarray(inputs["x"][0])              # [256, 1024]
    wq, wk, wv = inputs["wq"], inputs["wk"], inputs["wv"]  # [4, 64, 64]
    wo = inputs["wo"]                                      # [256, 256]
    w_off_dw = inputs["w_off_dw"][:, 0, :]                 # [64, 6]
    b_off_dw = inputs["b_off_dw"]                          # [64]
    w_off_proj = inputs["w_off_proj"]                      # [64]
    w1 = inputs["cpb_w1"][:, 0]                            # [64]
    b1 = inputs["cpb_b1"]                                  # [64]
    w2 = inputs["cpb_w2"]                                  # [64, 64]
    b2 = inputs["cpb_b2"]                                  # [64]
    w3 = inputs["cpb_w3"]                                  # [2, 64]

    f = np.float32
    b1col = np.concatenate([b1, b1]).astype(f)[:, None]
    w2bd = np.zeros((128, 128), f)
    w2bd[:64, :64] = w2.T
    w2bd[64:, 64:] = w2.T
    b2col = np.concatenate([b2, b2]).astype(f)[:, None]
    w3bd = np.zeros((128, 4), f)
    # col 2o+half carries w3[o] in hidden-half rows (t_sb row = 2o+half)
    w3bd[:64, 0] = w3[0]
    w3bd[64:, 1] = w3[0]
    w3bd[:64, 2] = w3[1]
    w3bd[64:, 3] = w3[1]
    w1dup = np.concatenate([w1, w1]).astype(f)[:, None]
    pofs = np.zeros((128, 1), f)
    pofs[:64, 0] = -1088.0
    pofs[64:, 0] = 0.0
    base_packed = np.zeros((128, 790), f)
    base_packed[:, 0:128] = w2bd
    base_packed[:, 128:256] = np.eye(128, dtype=f)
    base_packed[:, 776:777] = b1col
    base_packed[:, 777:778] = b2col
    base_packed[:, 781:785] = w3bd
    base_packed[:, 785:786] = w1dup
    base_packed[:, 786:787] = pofs

    in_maps = []
    for c in range(NCORES):
        g, qh = c // 2, c % 2
        xg = np.ascontiguousarray(x[64 * g:64 * (g + 1)], dtype=f)
        xgT = np.zeros((N + 1, DPG), f)
        xgT[0:N] = xg.T
        pk = base_packed.copy()
        pk[0:64, 256:320] = wq[g].T
        pk[0:64, 320:384] = wq[g].T * f(DH) ** f(-0.5)
        pk[0:64, 384:448] = wk[g].T
        pk[0:64, 448:512] = wv[g].T
        pk[0:64, 512:768] = wo[:, 64 * g:64 * (g + 1)].T
        pk[0:64, 768:774] = w_off_dw
        pk[64:128, 768:774] = w_off_dw
        pk[0:64, 774] = b_off_dw
        pk[64:128, 774] = b_off_dw
        pk[0:64, 787] = 0.5 * w_off_proj
        pk[64:128, 788] = 0.5 * w_off_proj
        pk[:, 778] = f(OFFT + QS * qh)
        m = {
            "xg": xg,
            "xq": np.ascontiguousarray(xg[:, QS * qh:QS * (qh + 1)]),
            "xgT": xgT,
            "packed": pk,
        }
        in_maps.append(m)
    return in_maps


def kernel(**inputs):
    if "nc" not in _CACHED:
        _CACHED["nc"] = build_nc()
    nc = _CACHED["nc"]
    in_maps = _shard_inputs(inputs)
    res = bass_utils.run_bass_kernel_spmd(nc, in_maps, core_ids=list(range(NCORES)))
    ys = [res.results[c]["y"] for c in range(NCORES)]
    bo = inputs["bo"]
    out = np.zeros((1, DIM, N), np.float32)
    for qh in range(2):
        acc = np.zeros((DIM, QS), np.float64)
        for g in range(G):
            acc += ys[2 * g + qh]
        out[0, :, QS * qh:QS * (qh + 1)] = (acc + bo.astype(np.float64)[:, None]).astype(np.float32)
    return out


# revision 51
# speedup vs baseline: 1.0161x; 1.0161x over previous
"""DeformableAttention1D on 8 TRN2 NeuronCores via Bass/Tile.

Sharding: core c handles offset-group g=c//2 (64 of 256 channels, 2 of 8 heads)
and query-half qh=c%2 (512 of 1024 positions). Each core computes its group's
offsets/gather/CPB/attention independently; the final output projection is
computed as a partial (wo sliced by group) and summed on the host (the
"all-reduce" of the output projection).

CPB bias via a 1-D table: bias[h,i,j] = F_h(grid_q[i] - vgrid[j]) where F_h is
the scalar CPB MLP. grid_q is uniform with step d=2/1023, so with a table
T_h[m] = F_h((m-1088)*d) the bias row for query block i is the contiguous
slice T_h[k_j + i] lerped with a per-j fraction w_j. The slice gather uses
dma_gather over a 64-phase replicated copy of T in DRAM (row stride must be
a multiple of 256B); the lerp is two diagonal-matrix PE matmuls accumulating
straight into the attention-logit PSUM together with q@k.

The kv deformable gather also uses dma_gather: the host passes x transposed
(xgT [1025, 64], row 1024 zeros); rows i0/i1 are gathered per j (indices
clamped via an unsigned-min to the zero row, which reproduces the reference's
zero padding), then lerped with per-partition weights and PE-transposed back.

The ACT engine is restricted to ONE table set (natural_log_exp_and_others:
Exp/Ln/Relu/Copy/Identity/Square) because runtime table swaps are broken in
this environment; tanh and erf(gelu) are composed from Exp + DVE ops.
"""
import os
import sys

sys.path.insert(0, "/opt/trn_rl_repo")

DEBUG = bool(os.environ.get("DEFORM_DEBUG"))

import numpy as np

import concourse.bacc as bacc
import concourse.bass as bass
import concourse.mybir as mybir
import concourse.tile as tile
import concourse.bass_utils as bass_utils
from concourse.ap import AP

F32 = mybir.dt.float32
F32R = mybir.dt.float32r
I32 = mybir.dt.int32
I16 = mybir.dt.int16
U32 = mybir.dt.uint32
BF16 = mybir.dt.bfloat16
AF = mybir.ActivationFunctionType
ALU = mybir.AluOpType

# model dims (hardcoded per problem spec)
DIM = 256
N = 1024
G = 4
HEADS = 8
DH = 32
NDS = 256          # downsampled kv positions
QS = 512           # queries per core
DPG = 64           # channels per group
OFF_K = 6
DS = 4             # downsample stride
OFF_SCALE = 4.0
NCORES = 8

# CPB bias table (bf16: 128-elem row stride = 256B)
OFFT = 1088        # table center: u = (tm - OFFT) * DELTA
MTAB = 2176        # table points
TPAD = 2304        # padded table length per head in DRAM
RSPAN = 2176       # elements per phase copy; rows per head = 17*128 = 2176
GROW = 640         # gathered row length (513 needed, 640 bf16 = 1280B)
NRROWS = 4348      # rows addressable in R (128-elem stride)
DELTA = 2.0 / 1023.0

# A&S 7.1.25 erf coefficients (3-term, |err| <= 2.5e-5)
ERF_P3 = 0.47047
ERF_A3 = [0.3480242, -0.0958798, 0.7478556]

_CACHED = {}


def _patch_act_tables():
    """Restrict activation-table selection to the single set that covers all
    ACT functions used by this kernel, so exactly one table load is emitted
    (runtime table swaps do not work in this environment)."""
    import concourse.hw_specs as hw_specs

    if getattr(bacc, "_deform_act_patch", False):
        return
    orig = hw_specs.get_activation_tables

    keep = "natural_log_exp_and_others"

    def patched(module_arch):
        tabs = orig(module_arch)
        keep_funcs = tabs[keep]
        out = {}
        for name, funcs in tabs.items():
            if name == keep:
                out[name] = funcs
            else:
                out[name] = funcs - keep_funcs
        return out

    bacc.get_activation_tables = patched
    bacc._deform_act_patch = True


def _erf_gelu(nc, sb, out_ap, x_ap, shape):
    """out = x * (1 + erf(x/sqrt(2))) via A&S 7.1.25 (0.5 folded into wproj)."""
    P, Nf = shape
    sq = sb.tile([P, Nf], F32, name="gelu_sq", tag="gelu_sq")
    nc.scalar.activation(sq[:], x_ap, AF.Square)
    e = sb.tile([P, Nf], F32, name="gelu_e", tag="gelu_e")
    # e = exp(-x^2/2)
    nc.scalar.activation(e[:], sq[:], AF.Exp, scale=-0.5)
    ax = sb.tile([P, Nf], F32, name="gelu_ax", tag="gelu_ax")
    # |x|/sqrt(2) = max(x, -x) * (1/sqrt2) folded into t below
    nc.vector.scalar_tensor_tensor(ax[:], x_ap, -1.0, x_ap, ALU.mult, ALU.max)
    t = sb.tile([P, Nf], F32, name="gelu_t", tag="gelu_t")
    nc.vector.tensor_scalar(t[:], ax[:], float(ERF_P3 / np.sqrt(2.0)), 1.0, ALU.mult, ALU.add)
    nc.vector.reciprocal(t[:], t[:])
    poly = sb.tile([P, Nf], F32, name="gelu_poly", tag="gelu_poly")
    # P(t) = ((a3 t + a2) t + a1) t
    nc.vector.tensor_scalar(poly[:], t[:], ERF_A3[2], ERF_A3[1], ALU.mult, ALU.add)
    nc.vector.tensor_tensor(poly[:], poly[:], t[:], ALU.mult)
    nc.vector.scalar_tensor_tensor(poly[:], poly[:], ERF_A3[0], t[:], ALU.add, ALU.mult)
    # erfa = 1 - poly*e   (= erf(|x|/sqrt2))
    erfa = sb.tile([P, Nf], F32, name="gelu_erfa", tag="gelu_erfa")
    nc.vector.tensor_tensor(erfa[:], poly[:], e[:], ALU.mult)
    nc.vector.tensor_scalar(erfa[:], erfa[:], -1.0, 1.0, ALU.mult, ALU.add)
    # copysign: erf(x) = sign(x)*erfa
    sgn = sb.tile([P, Nf], U32, name="gelu_sgn", tag="gelu_sgn")
    nc.vector.tensor_scalar(sgn[:], x_ap.bitcast(U32), 0x80000000, None, ALU.bitwise_and)
    erfs = sb.tile([P, Nf], F32, name="gelu_erfs", tag="gelu_erfs")
    nc.vector.tensor_tensor(erfs[:].bitcast(U32), erfa[:].bitcast(U32), sgn[:], ALU.bitwise_or)
    # out = (1 + erf) * x
    nc.vector.tensor_scalar(erfs[:], erfs[:], 1.0, None, ALU.add)
    nc.vector.tensor_tensor(out_ap, erfs[:], x_ap, ALU.mult)


def _tanh_rows(nc, sb, out_ap, x_ap, shape):
    """out = tanh(x) = sign(x) * (1 - 2/(exp(2*min(|x|,30))+1)) on small tiles."""
    P, Nf = shape
    ax = sb.tile([P, Nf], F32, name="th_ax", tag="th_ax")
    nc.vector.scalar_tensor_tensor(ax[:], x_ap, -1.0, x_ap, ALU.mult, ALU.max)
    nc.vector.tensor_scalar(ax[:], ax[:], 30.0, None, ALU.min)
    e = sb.tile([P, Nf], F32, name="th_e", tag="th_e")
    nc.scalar.activation(e[:], ax[:], AF.Exp, scale=2.0)
    nc.vector.tensor_scalar(e[:], e[:], 1.0, None, ALU.add)
    r = sb.tile([P, Nf], F32, name="th_r", tag="th_r")
    nc.vector.reciprocal(r[:], e[:])
    nc.vector.tensor_scalar(r[:], r[:], -2.0, 1.0, ALU.mult, ALU.add)
    sgn = sb.tile([P, Nf], U32, name="th_sgn", tag="th_sgn")
    nc.vector.tensor_scalar(sgn[:], x_ap.bitcast(U32), 0x80000000, None, ALU.bitwise_and)
    nc.vector.tensor_tensor(out_ap.bitcast(U32), r[:].bitcast(U32), sgn[:], ALU.bitwise_or)


def build_nc():
    _patch_act_tables()
    nc = bacc.Bacc("TRN2", target_bir_lowering=False, debug=False, num_devices=NCORES)

    din = {}

    def dt_in(name, shape):
        din[name] = nc.dram_tensor(name, shape, F32, kind="ExternalInput")
        return din[name]

    dt_in("xg", [DPG, N])
    dt_in("xq", [DPG, QS])
    dt_in("xgT", [N + 1, DPG])     # x group transposed; row N (=1024) is zeros
    dt_in("packed", [128, 790])
    y_out = nc.dram_tensor("y", [DIM, QS], F32, kind="ExternalOutput")
    dbg = {}
    if DEBUG:
        for nm, shp in [("dbg_q", [DPG, N]), ("dbg_ridx", [1, NDS]),
                        ("dbg_kv", [DPG, NDS]),
                        ("dbg_k", [DPG, NDS]), ("dbg_v", [DPG, NDS]),
                        ("dbg_T", [4, 1088]), ("dbg_S0", [128, 4 * GROW]),
                        ("dbg_avn", [DPG, QS])]:
            dbg[nm] = nc.dram_tensor(nm, shp, F32, kind="ExternalOutput")

    with tile.TileContext(nc) as tc:
        with (
            tc.tile_pool(name="const", bufs=1) as cst,
            tc.tile_pool(name="work", bufs=2) as wk,
            tc.tile_pool(name="rows", bufs=1) as rw,
            tc.tile_pool(name="persist", bufs=1) as pe_pool,
            tc.tile_pool(name="dramt", bufs=1, space="DRAM") as drp,
        ):
            # ---- input DMAs: packed first (gates the table build) ----
            packed = cst.tile([128, 790], F32, name="packed", tag="packed")
            nc.sync.dma_start(packed[:], din["packed"].ap())
            xg = cst.tile([DPG, N], F32, name="xg", tag="xg")
            nc.sync.dma_start(xg[:], din["xg"].ap())
            xq = cst.tile([DPG, QS], F32, name="xq", tag="xq")
            nc.sync.dma_start(xq[:], din["xq"].ap())
            w2bd = packed[:, 0:128]
            eyet = packed[:, 128:256]
            wqT = packed[0:DPG, 256:320]
            wqTs = packed[0:DPG, 320:384]
            wkT = packed[0:DPG, 384:448]
            wvT = packed[0:DPG, 448:512]
            woT = packed[0:DPG, 512:768]
            wdw = packed[0:DPG, 768:774]
            bodw = packed[0:DPG, 774:775]
            wproj_half = packed[0:DPG, 775:776]
            b1col = packed[:, 776:777]
            b2col = packed[:, 777:778]
            qbofft = packed[:, 778:779]     # OFFT + 512*qh
            w3bd = packed[:, 781:785]
            w1dup = packed[:, 785:786]      # [w1; w1] column
            pofs = packed[:, 786:787]       # -1088*(p<64), 0*(p>=64) column

            # ---- small constants / engine warmup ----
            ones_col = cst.tile([128, 1], F32, name="ones", tag="ones")
            nc.gpsimd.memset(ones_col[:], 1.0)
            warm = cst.tile([128, 1], F32, name="warm", tag="warm")
            nc.scalar.activation(warm[:], ones_col[:], AF.Relu)
            ones_colr = cst.tile([128, 1], F32R, name="onesr", tag="onesr")
            nc.vector.tensor_copy(ones_colr[:], ones_col[:])
            ones_rf = cst.tile([1, 32], F32, name="onesrf", tag="onesrf")
            nc.gpsimd.memset(ones_rf[:], 1.0)
            ones_rowr = cst.tile([1, 32], F32R, name="onesrw", tag="onesrw")
            nc.vector.tensor_copy(ones_rowr[:], ones_rf[:])

            # persistent tiles
            k_sb = pe_pool.tile([DPG, NDS], F32R, name="k_sb", tag="k_sb")
            qs_sb = pe_pool.tile([DPG, QS], F32R, name="qs_sb", tag="qs_sb")
            vT = [pe_pool.tile([128, DPG], F32R, name=f"vT{H}", tag=f"vT{H}") for H in range(2)]
            avn = pe_pool.tile([DPG, QS], F32R, name="avn", tag="avn")
            gath = pe_pool.tile([128, 4 * GROW], BF16, name="gath", tag="gath")
            gkv = pe_pool.tile([128, 4 * DPG], F32, name="gkv", tag="gkv")
            diag_w = [pe_pool.tile([128, 128], BF16, name=f"diag_w{H}", tag=f"diag_w{H}")
                      for H in range(2)]
            diag_1w = [pe_pool.tile([128, 128], BF16, name=f"diag_1w{H}", tag=f"diag_1w{H}")
                       for H in range(2)]

            t_dram = drp.tile([2, TPAD], BF16, name="t_dram", tag="t_dram")
            r_dram = drp.tile([2, 128 * RSPAN], BF16, name="r_dram", tag="r_dram")

            # ================= CPB table build (offset-independent) =========
            with (
                tc.tile_pool(name="tblw", bufs=1) as tbw,
                tc.tile_pool(name="psT", bufs=1, space="PSUM") as psT,
                tc.tile_pool(name="psT2", bufs=1, space="PSUM") as psT2,
                tc.tile_pool(name="psA", bufs=2, space="PSUM") as psA,
                tc.tile_pool(name="psA1", bufs=1, space="PSUM") as psA1,
            ):
                # --- phi(u) in [128 = 2x64 hidden, 1088] broadcast layout ---
                io_t = tbw.tile([128, 1088], F32, name="io_t", tag="io_t")
                nc.gpsimd.iota(io_t[:], pattern=[[1, 1088]], base=0, channel_multiplier=0,
                               allow_small_or_imprecise_dtypes=True)
                xgr = cst.tile([DPG, N], F32R, name="xgr", tag="xgr")
                nc.vector.tensor_copy(xgr[:], xg[:])
                xqr = cst.tile([DPG, QS], F32R, name="xqr", tag="xqr")
                nc.vector.tensor_copy(xqr[:], xq[:])
                wqTr = cst.tile([DPG, DPG], F32R, name="wqTr", tag="wqTr")
                nc.vector.tensor_copy(wqTr[:], wqT)
                wqTsr = cst.tile([DPG, DPG], F32R, name="wqTsr", tag="wqTsr")
                nc.vector.tensor_copy(wqTsr[:], wqTs)
                u_t = tbw.tile([128, 1088], F32, name="u_t", tag="u_t")
                nc.vector.tensor_scalar(u_t[:], io_t[:], pofs, DELTA, ALU.add, ALU.mult)
                au_t = tbw.tile([128, 1088], F32, name="au_t", tag="au_t")
                nc.vector.scalar_tensor_tensor(au_t[:], u_t[:], -1.0, u_t[:], ALU.mult, ALU.max)
                sg_t = tbw.tile([128, 1088], U32, name="sg_t", tag="sg_t")
                nc.vector.tensor_scalar(sg_t[:], u_t[:].bitcast(U32), 0x80000000, None, ALU.bitwise_and)
                # f32r weight copies (DVE, cheap, off the phi critical path)
                w2bdr = cst.tile([128, 128], F32R, name="w2bdr", tag="w2bdr")
                nc.vector.tensor_copy(w2bdr[:], w2bd)
                w3bdr = cst.tile([128, 4], F32R, name="w3bdr", tag="w3bdr")
                nc.vector.tensor_copy(w3bdr[:], w3bd)
                woTr = cst.tile([DPG, DIM], F32R, name="woTr", tag="woTr")
                nc.vector.tensor_copy(woTr[:], woT)
                wkTr = cst.tile([DPG, DPG], F32R, name="wkTr", tag="wkTr")
                nc.vector.tensor_copy(wkTr[:], wkT)
                wvTr = cst.tile([DPG, DPG], F32R, name="wvTr", tag="wvTr")
                nc.vector.tensor_copy(wvTr[:], wvT)
                ln_t = tbw.tile([128, 1088], F32, name="ln_t", tag="ln_t")
                nc.scalar.activation(ln_t[:], au_t[:], AF.Ln, bias=1.0)
                phi_t = tbw.tile([128, 1088], F32, name="phi_t", tag="phi_t")
                nc.vector.tensor_tensor(phi_t[:].bitcast(U32), ln_t[:].bitcast(U32), sg_t[:], ALU.bitwise_or)
                h1_t = tbw.tile([128, 1088], F32R, name="h1_t", tag="h1_t")
                nc.scalar.activation(h1_t[:], phi_t[:], AF.Relu, bias=b1col, scale=w1dup)

                # --- q matmuls early on PE (before the table MLP matmuls) ---
                # q packed: q_pad[c + 64h, t] = q[c, 512h + t - 1]
                q_pad = pe_pool.tile([128, 516], F32, name="q_pad", tag="q_pad")
                nc.gpsimd.memset(q_pad[:], 0.0)
                pqh = []
                for h in range(2):
                    pq = psA.tile([DPG, QS], F32, name="pA512", tag="pA512")
                    nc.tensor.matmul(pq[:], wqTr[:], xgr[:, h * QS:(h + 1) * QS])
                    nc.scalar.copy(q_pad[64 * h:64 * (h + 1), 1:513], pq[:])
                    pqh.append(pq)
                nc.scalar.copy(q_pad[0:64, 513:514], pqh[1][:, 0:1])
                nc.scalar.copy(q_pad[64:128, 0:1], pqh[0][:, 511:512])
                pqs = psA.tile([DPG, QS], F32, name="pA512", tag="pA512")
                nc.tensor.matmul(pqs[:], wqTsr[:], xqr[:])
                nc.scalar.copy(qs_sb[:], pqs[:])

                # --- table MLP layers 2+3 in 512-col blocks ---
                t_sb = tbw.tile([4, 1088], BF16, name="t_sb", tag="t_sb")
                for blo, bhi in ((0, 512), (512, 1024), (1024, 1088)):
                    bw = bhi - blo
                    pre2 = psT.tile([128, bw], F32, name="pre2", tag="pre2")
                    nc.tensor.matmul(pre2[:], w2bdr[:], h1_t[:, blo:bhi])
                    h2_t = wk.tile([128, bw], F32R, name="h2_t", tag="h2_t")
                    nc.scalar.activation(h2_t[:], pre2[:], AF.Relu, bias=b2col)
                    pt3 = psT2.tile([4, bw], F32, name="pt3", tag="pt3")
                    nc.tensor.matmul(pt3[:], w3bdr[:], h2_t[:])
                    nc.scalar.copy(t_sb[:, blo:bhi], pt3[:])
                # T rows (2*half + o) -> t_dram[o, 1088*half + q]; zero pad tail
                zrow = tbw.tile([1, 128], F32, name="zrow", tag="zrow")
                nc.gpsimd.memset(zrow[:], 0.0)
                for o in range(2):
                    # w3bd cols reordered so t_sb row 2o+half = (o, half)
                    nc.sync.dma_start(t_dram[:][o, 0:MTAB].rearrange("(h q) -> h q", h=2),
                                      t_sb[2 * o:2 * o + 2, :])
                    nc.sync.dma_start(t_dram[:][o, MTAB:TPAD],
                                      zrow[:, 0:(TPAD - MTAB) // 2].bitcast(BF16))
                # R: 64 overlapping phase copies per head, one 3-dim DMA
                tv = t_dram[:]
                rsrc = AP(tv.tensor, tv.offset, [[TPAD, 2], [1, 128], [1, RSPAN]])
                nc.sync.dma_start(r_dram[:].rearrange("o (c e) -> o c e", c=128), rsrc)
                if DEBUG:
                    nc.sync.dma_start(dbg["dbg_T"].ap(), t_sb[:].bitcast(F32))

                # ===== offsets chain (packed: [128, 128], rows [2, 128]) =====
                # depthwise strided conv; partition block h covers j in
                # [128h, 128h+128); tap kk reads q_pad[:, kk + 4*jj]
                wdw_d = packed[:, 768:774]
                bodw_d = packed[:, 774:775]
                wproj2 = packed[:, 787:789]
                acc = wk.tile([128, 128], F32, name="conv_acc", tag="conv_acc")
                acc2 = wk.tile([128, 128], F32, name="conv_acc2", tag="conv_acc2")
                nc.vector.tensor_scalar(
                    acc[:], q_pad[:, 0:509:DS], wdw_d[:, 0:1], bodw_d, ALU.mult, ALU.add)
                nc.vector.tensor_scalar(
                    acc2[:], q_pad[:, 1:510:DS], wdw_d[:, 1:2], None, ALU.mult)
                for kk in range(2, OFF_K):
                    dst = acc if kk % 2 == 0 else acc2
                    nc.vector.scalar_tensor_tensor(
                        dst[:], q_pad[:, kk:kk + 509:DS], wdw_d[:, kk:kk + 1], dst[:],
                        ALU.mult, ALU.add)
                nc.vector.tensor_tensor(acc[:], acc[:], acc2[:], ALU.add)

                gl = wk.tile([128, 128], F32, name="gelu_out", tag="gelu_out")
                _erf_gelu(nc, wk, gl[:], acc[:], [128, 128])

                # proj rows: pproj[h, jj] = sum_c 0.5*wproj[c]*gl[c+64h, jj]
                pproj = psA1.tile([2, 128], F32, name="pproj", tag="pproj")
                nc.tensor.matmul(pproj[:], wproj2, gl[:])
                proj_sb = rw.tile([2, 128], F32, name="proj_sb", tag="proj_sb")
                nc.scalar.copy(proj_sb[:], pproj[:])
                th = rw.tile([2, 128], F32, name="th", tag="th")
                _tanh_rows(nc, rw, th[:], proj_sb[:], [2, 128])

                iotaj = rw.tile([2, 128], F32, name="iotaj", tag="iotaj")
                nc.gpsimd.iota(iotaj[:], pattern=[[1, 128]], base=0, channel_multiplier=128,
                               allow_small_or_imprecise_dtypes=True)
                vgrid = rw.tile([2, 128], F32, name="vgrid", tag="vgrid")
                nc.vector.scalar_tensor_tensor(vgrid[:], th[:], OFF_SCALE, iotaj[:], ALU.mult, ALU.add)

                # ---- kv gather index chain (Pool helps, parallel with CPB) ----
                ppix = rw.tile([2, 128], F32, name="ppix", tag="ppix")
                nc.vector.tensor_scalar(ppix[:], vgrid[:], float(N / (NDS - 1)), -0.5, ALU.mult, ALU.add)
                pi = rw.tile([2, 128], I32, name="pi", tag="pi")
                nc.vector.tensor_copy(pi[:], ppix[:])
                pc = rw.tile([2, 128], F32, name="pc", tag="pc")
                nc.vector.tensor_copy(pc[:], pi[:])
                i01 = rw.tile([2, 256], F32, name="i01", tag="i01")
                w0r = rw.tile([2, 128], F32, name="w0r", tag="w0r")
                w1r = rw.tile([2, 128], F32, name="w1r", tag="w1r")
                gtp = rw.tile([2, 128], F32, name="gtp", tag="gtp")
                i0f = rw.tile([2, 128], F32, name="i0f", tag="i0f")
                nc.vector.tensor_tensor(gtp[:], pc[:], ppix[:], ALU.is_gt)
                nc.gpsimd.tensor_sub(i0f[:], pc[:], gtp[:])
                nc.gpsimd.tensor_sub(w1r[:], ppix[:], i0f[:])
                nc.vector.tensor_scalar(w0r[:], w1r[:], -1.0, 1.0, ALU.mult, ALU.add)
                # clamp OOB to the zero row (1024): unsigned-min on f32 bits
                nc.gpsimd.tensor_scalar_min(i01[:, 0:128].bitcast(U32), i0f[:].bitcast(U32),
                                            0x44800000)
                i1f = rw.tile([2, 128], F32, name="i1f", tag="i1f")
                nc.gpsimd.tensor_scalar_add(i1f[:], i0f[:], 1.0)
                nc.gpsimd.tensor_scalar_min(i01[:, 128:256].bitcast(U32), i1f[:].bitcast(U32),
                                            0x44800000)

                # ---- CPB table index chain (DVE) ----
                ridx = rw.tile([2, 128], F32, name="ridx", tag="ridx")
                nc.vector.tensor_scalar(ridx[:], vgrid[:], float(-1023.0 / 255.0), qbofft[0:2, 0:1],
                                        ALU.mult, ALU.add)
                ki = rw.tile([2, 128], I32, name="ki", tag="ki")
                nc.vector.tensor_copy(ki[:], ridx[:])
                kc = rw.tile([2, 128], F32, name="kc", tag="kc")
                nc.vector.tensor_copy(kc[:], ki[:])
                gtk = rw.tile([2, 128], F32, name="gtk", tag="gtk")
                nc.vector.tensor_tensor(gtk[:], kc[:], ridx[:], ALU.is_gt)
                kf = rw.tile([2, 128], F32, name="kf", tag="kf")
                nc.vector.tensor_tensor(kf[:], kc[:], gtk[:], ALU.subtract)
                wfr = rw.tile([2, 128], F32, name="wfr", tag="wfr")
                nc.vector.tensor_tensor(wfr[:], ridx[:], kf[:], ALU.subtract)
                kii = rw.tile([2, 128], I32, name="kii", tag="kii")
                nc.vector.tensor_copy(kii[:], kf[:])
                # r = 17*(k & 127) + (k >> 7), +RSPAN rows for head 1
                q64 = rw.tile([2, 128], I32, name="q64", tag="q64")
                nc.vector.tensor_scalar(q64[:], kii[:], 7, None, ALU.arith_shift_right)
                cph = rw.tile([2, 128], I32, name="cph", tag="cph")
                nc.vector.tensor_scalar(cph[:], kii[:], 127, None, ALU.bitwise_and)
                ri = rw.tile([2, 128], I32, name="ri", tag="ri")
                nc.vector.scalar_tensor_tensor(ri[:], cph[:], 17, q64[:], ALU.mult, ALU.add)
                rb0 = rw.tile([2, 128], F32, name="rb0", tag="rb0")
                nc.vector.tensor_copy(rb0[:], ri[:])
                rb1 = rw.tile([2, 128], F32, name="rb1", tag="rb1")
                nc.vector.tensor_scalar(rb1[:], rb0[:], float(RSPAN), None, ALU.add)

                # ---- wrap index sets [2, 128]x2 -> [32, 16] -> [32, 128]
                # (8x replicated) -> PE transpose -> [128, 32] -> int16; the
                # two DMAs per set ride the ACT hwdge queue
                def wrap_a(srcs, nm):
                    sw16 = rw.tile([32, 16], F32, name=f"sw16{nm}", tag=f"sw16{nm}")
                    for b, s in enumerate(srcs):
                        nc.scalar.dma_start(sw16[16 * b:16 * (b + 1), :],
                                            s.rearrange("p (s e) -> p s e", s=8))
                    sw = rw.tile([32, 128], F32, name=f"sw{nm}", tag=f"sw{nm}")
                    s16 = sw16[:]
                    sbc = AP(s16.tensor, s16.offset, [list(s16.ap[0]), [0, 8], [1, 16]])
                    nc.scalar.dma_start(sw[:].rearrange("p (r e) -> p r e", r=8), sbc)
                    return sw

                sw_kv = wrap_a([i01[:, 0:128], i01[:, 128:256]], "kv")
                sw_cpb = wrap_a([rb0[:], rb1[:]], "cpb")

                def wrap_b(sw, nm):
                    ptw = psA.tile([128, 128], F32, name=f"ptw{nm}", tag="ptp")
                    nc.tensor.transpose(ptw[:, 0:32], sw[:], eyet[0:32, 0:32])
                    rwp = rw.tile([128, 32], F32, name=f"rw{nm}", tag=f"rw{nm}")
                    nc.vector.tensor_copy(rwp[:], ptw[:, 0:32])
                    ix = rw.tile([128, 32], I16, name=f"ix{nm}", tag=f"ix{nm}")
                    nc.vector.tensor_copy(ix[:], rwp[:])
                    return ix

                ix_kv = wrap_b(sw_kv, "kv")
                ix_cpb = wrap_b(sw_cpb, "cpb")

                # ---- the two gathers ----
                xtv = din["xgT"].ap().flatten()
                ksrc = AP(xtv.tensor, xtv.offset, [[64, N + 1], [1, DPG]])
                nc.gpsimd.dma_gather(gkv[:].rearrange("p (b e) -> p b e", b=4), ksrc,
                                     ix_kv[:], 2 * NDS, 2 * NDS, DPG, elem_step=64)
                rv = r_dram[:].flatten()
                gsrc = AP(rv.tensor, rv.offset, [[128, NRROWS], [1, GROW]])
                nc.gpsimd.dma_gather(gath[:].rearrange("p (b e) -> p b e", b=4), gsrc,
                                     ix_cpb[:], 2 * NDS, 2 * NDS, GROW, elem_step=128)

                # ---- lerp weights to per-partition columns (one PE transpose each) ----
                def cols2(row2_ap, nm):
                    ptv = psA.tile([128, 128], F32, name=f"ptv{nm}", tag="ptp")
                    nc.tensor.transpose(ptv[:, 0:2], row2_ap, eyet[0:2, 0:2])
                    col = rw.tile([128, 2], F32, name=f"c{nm}", tag=f"c{nm}")
                    nc.vector.tensor_copy(col[:], ptv[:, 0:2])
                    return col

                w0c2 = cols2(w0r[:], "w0")
                w1c2 = cols2(w1r[:], "w1")
                wfc2 = cols2(wfr[:], "wf")
                w0c = [w0c2[:, 0:1], w0c2[:, 1:2]]
                w1c = [w1c2[:, 0:1], w1c2[:, 1:2]]
                for H in range(2):
                    nc.vector.tensor_scalar(diag_w[H][:], eyet, wfc2[:, H:H + 1], None, ALU.mult)
                    w1m = rw.tile([128, 1], F32, name=f"w1m{H}", tag=f"w1m{H}")
                    nc.vector.tensor_scalar(w1m[:], wfc2[:, H:H + 1], -1.0, 1.0, ALU.mult, ALU.add)
                    nc.vector.tensor_scalar(diag_1w[H][:], eyet, w1m[:], None, ALU.mult)

                # ---- kv lerp + transpose back to [c, j] ----
                kvTw = wk.tile([128, 128], F32, name="kvTw", tag="kvTw")
                for H in range(2):
                    tmp = wk.tile([128, DPG], F32, name="kvt_t", tag="kvt_t")
                    nc.vector.tensor_scalar(tmp[:], gkv[:, DPG * H:DPG * (H + 1)],
                                            w0c[H], None, ALU.mult)
                    nc.vector.scalar_tensor_tensor(kvTw[:, DPG * H:DPG * (H + 1)],
                                                   gkv[:, 2 * DPG + DPG * H:2 * DPG + DPG * (H + 1)],
                                                   w1c[H], tmp[:], ALU.mult, ALU.add)
                kv = wk.tile([DPG, NDS], F32R, name="kv", tag="kv")
                for H in range(2):
                    ptk = psA.tile([128, 128], F32, name="ptk", tag="ptp")
                    nc.tensor.transpose(ptk[0:DPG, :], kvTw[:, DPG * H:DPG * (H + 1)],
                                        eyet)
                    nc.vector.tensor_copy(kv[:, 128 * H:128 * (H + 1)], ptk[0:DPG, :])
                if DEBUG:
                    nc.sync.dma_start(dbg["dbg_kv"].ap(), kv[:].bitcast(F32))

                pk = psA1.tile([DPG, NDS], F32, name="pA256", tag="pA256")
                nc.tensor.matmul(pk[:], wkTr[:], kv[:])
                nc.scalar.copy(k_sb[:], pk[:])
                pv = psA1.tile([DPG, NDS], F32, name="pA256", tag="pA256")
                nc.tensor.matmul(pv[:], wvTr[:], kv[:])
                v_sb = wk.tile([DPG, NDS], F32, name="v_sb", tag="v_sb")
                nc.scalar.copy(v_sb[:], pv[:])
                if DEBUG:
                    nc.sync.dma_start(dbg["dbg_k"].ap(), k_sb[:].bitcast(F32))
                    nc.sync.dma_start(dbg["dbg_v"].ap(), v_sb[:])

                for H in range(2):
                    pt = psA.tile([128, 128], F32, name="ptvv", tag="ptp")
                    nc.tensor.transpose(pt[:, 0:DPG], v_sb[:, H * 128:(H + 1) * 128], eyet[0:DPG, 0:DPG])
                    nc.vector.tensor_copy(vT[H][:], pt[:, 0:DPG])

            # ============ attention ============
            with (
                tc.tile_pool(name="psE", bufs=2, space="PSUM") as psE,
                tc.tile_pool(name="psE1", bufs=1, space="PSUM") as psE1,
                tc.tile_pool(name="psE2", bufs=1, space="PSUM") as psE2,
            ):
                for h in range(2):
                    expT = []
                    for H in range(2):
                        psim = psE.tile([128, QS], F32, name="psim", tag="psim")
                        g0 = gath[:, (2 * h + H) * GROW:(2 * h + H) * GROW + 512]
                        g1 = gath[:, (2 * h + H) * GROW + 1:(2 * h + H) * GROW + 513]
                        nc.tensor.matmul(psim[:], diag_1w[H][:], g0, start=True, stop=False)
                        nc.tensor.matmul(psim[:], diag_w[H][:], g1, start=False, stop=False)
                        nc.tensor.matmul(
                            psim[:], k_sb[32 * h:32 * (h + 1), H * 128:(H + 1) * 128],
                            qs_sb[32 * h:32 * (h + 1), :], start=False, stop=True)
                        et = wk.tile([128, QS], F32R, name="expT", tag="expT")
                        nc.scalar.activation(et[:], psim[:], AF.Exp)
                        expT.append(et)

                    psum_s = psE1.tile([1, QS], F32, name="psum_s", tag="psum_s")
                    for H in range(2):
                        nc.tensor.matmul(psum_s[:], ones_colr[:], expT[H][:],
                                         start=(H == 0), stop=(H == 1))
                    rs = rw.tile([1, QS], F32R, name="rs", tag="rs")
                    with nc.allow_low_precision(reason="f32r 1/sum feeds f32r PE broadcast"):
                        nc.vector.reciprocal(rs[:], psum_s[:])
                    # broadcast 1/sum to 32 partitions via PE (K=1 matmul)
                    prsb = psE1.tile([32, QS], F32, name="prsb", tag="prsb")
                    nc.tensor.matmul(prsb[:], ones_rowr[:], rs[:])

                    pav = psE2.tile([32, QS], F32, name="pav", tag="pav")
                    for H in range(2):
                        nc.tensor.matmul(pav[:], vT[H][:, 32 * h:32 * (h + 1)], expT[H][:],
                                         start=(H == 0), stop=(H == 1))
                    # pav -> SBUF early (ACT, off the recip chain); avn then has
                    # only one PSUM operand (prsb)
                    pav_sb = wk.tile([32, QS], F32, name="pav_sb", tag="pav_sb")
                    nc.scalar.copy(pav_sb[:], pav[:])
                    nc.vector.tensor_tensor(avn[32 * h:32 * (h + 1), :], pav_sb[:], prsb[:], ALU.mult)
                if DEBUG:
                    nc.sync.dma_start(dbg["dbg_avn"].ap(), avn[:].bitcast(F32))

                for m in range(2):
                    py = psE.tile([128, QS], F32, name="py", tag="py")
                    nc.tensor.matmul(py[:], woTr[0:32, m * 128:(m + 1) * 128], avn[0:32, :],
                                     start=True, stop=False)
                    nc.tensor.matmul(py[:], woTr[32:64, m * 128:(m + 1) * 128], avn[32:64, :],
                                     start=False, stop=True)
                    y_sb = wk.tile([128, QS], F32, name="y_sb", tag="y_sb")
                    nc.scalar.copy(y_sb[:], py[:])
                    nc.sync.dma_start(y_out.ap()[m * 128:(m + 1) * 128, :], y_sb[:])

    nc.compile()
    return nc


def _shard_inputs(inputs):
    """Build the 8 per-core input maps from the full inputs."""
    x = np.ascontiguousarray(inputs["x"][0])              # [256, 1024]
    wq, wk, wv = inputs["wq"], inputs["wk"], inputs["wv"]  # [4, 64, 64]
    wo = inputs["wo"]                                      # [256, 256]
    w_off_dw = inputs["w_off_dw"][:, 0, :]                 # [64, 6]
    b_off_dw = inputs["b_off_dw"]                          # [64]
    w_off_proj = inputs["w_off_proj"]                      # [64]
    w1 = inputs["cpb_w1"][:, 0]                            # [64]
    b1 = inputs["cpb_b1"]                                  # [64]
    w2 = inputs["cpb_w2"]                                  # [64, 64]
    b2 = inputs["cpb_b2"]                                  # [64]
    w3 = inputs["cpb_w3"]                                  # [2, 64]

    f = np.float32
    b1col = np.concatenate([b1, b1]).astype(f)[:, None]
    w2bd = np.zeros((128, 128), f)
    w2bd[:64, :64] = w2.T
    w2bd[64:, 64:] = w2.T
    b2col = np.concatenate([b2, b2]).astype(f)[:, None]
    w3bd = np.zeros((128, 4), f)
    # col 2o+half carries w3[o] in hidden-half rows (t_sb row = 2o+half)
    w3bd[:64, 0] = w3[0]
    w3bd[64:, 1] = w3[0]
    w3bd[:64, 2] = w3[1]
    w3bd[64:, 3] = w3[1]
    w1dup = np.concatenate([w1, w1]).astype(f)[:, None]
    pofs = np.zeros((128, 1), f)
    pofs[:64, 0] = -1088.0
    pofs[64:, 0] = 0.0
    base_packed = np.zeros((128, 790), f)
    base_packed[:, 0:128] = w2bd
    base_packed[:, 128:256] = np.eye(128, dtype=f)
    base_packed[:, 776:777] = b1col
    base_packed[:, 777:778] = b2col
    base_packed[:, 781:785] = w3bd
    base_packed[:, 785:786] = w1dup
    base_packed[:, 786:787] = pofs

    in_maps = []
    for c in range(NCORES):
        g, qh = c // 2, c % 2
        xg = np.ascontiguousarray(x[64 * g:64 * (g + 1)], dtype=f)
        xgT = np.zeros((N + 1, DPG), f)
        xgT[0:N] = xg.T
        pk = base_packed.copy()
        pk[0:64, 256:320] = wq[g].T
        pk[0:64, 320:384] = wq[g].T * f(DH) ** f(-0.5)
        pk[0:64, 384:448] = wk[g].T
        pk[0:64, 448:512] = wv[g].T
        pk[0:64, 512:768] = wo[:, 64 * g:64 * (g + 1)].T
        pk[0:64, 768:774] = w_off_dw
        pk[64:128, 768:774] = w_off_dw
        pk[0:64, 774] = b_off_dw
        pk[64:128, 774] = b_off_dw
        pk[0:64, 787] = 0.5 * w_off_proj
        pk[64:128, 788] = 0.5 * w_off_proj
        pk[:, 778] = f(OFFT + QS * qh)
        m = {
            "xg": xg,
            "xq": np.ascontiguousarray(xg[:, QS * qh:QS * (qh + 1)]),
            "xgT": xgT,
            "packed": pk,
        }
        in_maps.append(m)
    return in_maps


def kernel(**inputs):
    if "nc" not in _CACHED:
        _CACHED["nc"] = build_nc()
    nc = _CACHED["nc"]
    in_maps = _shard_inputs(inputs)
    res = bass_utils.run_bass_kernel_spmd(nc, in_maps, core_ids=list(range(NCORES)))
    ys = [res.results[c]["y"] for c in range(NCORES)]
    bo = inputs["bo"]
    out = np.zeros((1, DIM, N), np.float32)
    for qh in range(2):
        acc = np.zeros((DIM, QS), np.float64)
        for g in range(G):
            acc += ys[2 * g + qh]
        out[0, :, QS * qh:QS * (qh + 1)] = (acc + bo.astype(np.float64)[:, None]).astype(np.float32)
    return out
